# revision 31
# baseline (speedup 1.0000x reference)
"""DiffusionConv (4x GCN message passing) Trainium2 kernel, 8-core SPMD.

Strategy: shard destination nodes across 8 cores (3750 each). Each core:
  - gathers source-node feature rows (fp16) for its edges via dma_gather,
    one large gather per (adjacency, 128-dst window) to amortize SWDGE
    descriptor-generation overhead; edges pre-sorted by destination and
    padded per window,
  - builds the banded [128e x 128d] edge-weight matrices on-chip with a
    single fused DVE pass per 128-edge chunk:
    st[e, d] = (iota[d] == dstcol[e]) * ew[e]  (tensor_scalar, two ops),
  - aggregates with one matmul per chunk accumulating in PSUM,
  - applies the 32x32 weight matrices via PE transpose + block-diagonal
    matmuls, interleaved per-window with the next window's aggregation
    (1-window software pipeline) so the gather DMA stream never idles,
  - adds bias, writes f32 output.
No cross-core communication: each core reads a full replica of x.
"""
import sys, os
for p in ('/opt/trn_rl_repo', '/root/.axon_site/_ro/trn_rl_repo'):
    if os.path.isdir(p) and p not in sys.path:
        sys.path.insert(0, p)

import numpy as np
import ml_dtypes

N = 30000
C = 32
T = 12
ES = C * T          # 384, feature row width
E = 480000
NCORES = 8
ND = N // NCORES    # 3750 dst nodes per core
NDP = 3840          # padded to 30 windows of 128
WIN = 128           # dst window width
NWIN = NDP // WIN   # 30
ADJS = ("fwd1", "fwd2", "bck1", "bck2")

bf16 = np.float16 if os.environ.get("K_DT", "fp16") == "fp16" else ml_dtypes.bfloat16
X8 = os.environ.get("K_X8", "0") == "1"   # gather x rows in fp8e4m3 (512B padded)
XROW = 512 if X8 else ES                  # gathered row width (elements)
W64 = os.environ.get("K_W64", "0") == "1" # 64-wide aggregation sub-windows
WAGG = 64 if W64 else 128                 # aggregation window width (dsts)
NWAGG = NDP // WAGG                       # aggregation windows per core


def _prep_edges(ei, ew):
    """Split one adjacency's edges by destination core and sort by dst."""
    src = np.asarray(ei[0]).astype(np.int64)
    dst = np.asarray(ei[1]).astype(np.int64)
    w = np.asarray(ew).astype(np.float32)
    core = dst // ND
    out = []
    for k in range(NCORES):
        sel = core == k
        dl = dst[sel] - k * ND
        s = src[sel]
        wv = w[sel]
        order = np.argsort(dl, kind="stable")
        dl = dl[order]
        s = s[order]
        wv = wv[order]
        win = dl // WIN
        bounds = np.searchsorted(win, np.arange(NWIN + 1))
        counts = np.diff(bounds)
        out.append((s, dl, wv, bounds, counts))
    return out


def _build_adj_inputs(per_core):
    """Uniform-shape device inputs for one adjacency: gather idx tiles and
    per-edge (dst-local column, weight) tiles; per-window chunk counts are
    shared across cores (max)."""
    ncw = np.maximum(1, -(-np.stack([pc[4] for pc in per_core], 0).max(0) // 128))
    nch = int(ncw.sum())
    epad = nch * 128
    winid = np.repeat(np.arange(NWIN), ncw * 128)
    gis, dlcs = [], []
    for k in range(NCORES):
        s, dl, wv, bounds, counts = per_core[k]
        srcp = np.zeros(epad, np.int64)
        dlp = np.zeros(epad, np.int64)
        ewp = np.zeros(epad, np.float32)
        off = 0
        for w in range(NWIN):
            c = int(counts[w])
            lo, hi = int(bounds[w]), int(bounds[w + 1])
            srcp[off:off + c] = s[lo:hi]
            dlp[off:off + c] = dl[lo:hi]
            ewp[off:off + c] = wv[lo:hi]
            dlp[off + c:off + int(ncw[w]) * 128] = w * WIN  # pads (ew 0)
            off += int(ncw[w]) * 128
        # gather idx, wrapped layout [128, epad//16]: idx i -> [i%16, i//16]
        gi = np.tile(srcp.astype(np.int16).reshape(-1, 16).T, (8, 1))
        col = (dlp - winid * WIN).astype(np.float32)   # 0..127
        # device layout [128 partitions = edge-in-chunk, nch], float32
        dlc = col.reshape(nch, 128).T.astype(bf16)
        ewc = ewp.reshape(nch, 128).T.astype(bf16)
        gis.append(np.ascontiguousarray(gi))
        dlcs.append(np.ascontiguousarray(np.stack([dlc, ewc], 1)))  # [128, 2, nch]
    return ncw.astype(np.int64), gis, dlcs


def _build_B(Ws):
    """Block-diagonal weight tiles B[a][k]: [128, 384] bf16.
    B[a,k][p, c'*12 + t] = W_a[c, c'] with (t, c) = divmod(128k + p, 32)."""
    B = np.zeros((4, 3, 128, ES), np.float32)
    for a in range(4):
        Wa = np.asarray(Ws[a]).astype(np.float32)
        for k in range(3):
            phi = 128 * k + np.arange(128)
            t = phi // 32
            c = phi % 32
            B[a, k, np.arange(128)[:, None], np.arange(32)[None, :] * 12 + t[:, None]] = Wa[c, :]
    return B.astype(bf16)


_CACHE = {}
LAST_RESULTS = None
LAST_NC = None
LAST_NCWS = None
LAST_INMAPS = None


def _get_program(ncws, rep=1, ablate=()):
    """Build (and cache) the Bass program for given per-adjacency window
    chunk counts. ncws: tuple of 4 tuples of NWIN ints. rep>1 repeats the
    pipeline (device-time measurement via slope); ablate disables stages
    for profiling ("nogather", "noaggmm", "nowphase", "nostbuild").
    """
    key = (ncws, rep, tuple(sorted(ablate)))
    if key in _CACHE:
        return _CACHE[key]

    import concourse.mybir as mybir
    import concourse.tile as tile
    from concourse import bacc
    from concourse.masks import make_identity

    DT = (mybir.dt.float16 if os.environ.get("K_DT", "fp16") == "fp16"
          else mybir.dt.bfloat16)
    F32 = mybir.dt.float32
    XDT = mybir.dt.float8e4 if X8 else DT   # dtype of gathered rows / st
    nchs = [int(sum(ncw)) for ncw in ncws]
    choffs = [[int(sum(ncw[:w])) for w in range(NWIN)] for ncw in ncws]
    max_nch = max(int(v) for ncw in ncws for v in ncw)

    MSGBUFS = int(os.environ.get('K_MSGBUFS', 6))
    STBUFS = int(os.environ.get('K_STBUFS', 8))

    nc = bacc.Bacc("TRN2", target_bir_lowering=False, debug=False,
                   num_devices=NCORES, num_swdge_queues=4)
    x_d = nc.dram_tensor("xrows", [N, XROW], XDT, kind="ExternalInput")
    gi_d = [nc.dram_tensor(f"gi_{a}", [128, nchs[ai] * 8], mybir.dt.int16,
                           kind="ExternalInput") for ai, a in enumerate(ADJS)]
    dew_d = [nc.dram_tensor(f"dew_{a}", [128, 2, nchs[ai]], DT,
                            kind="ExternalInput") for ai, a in enumerate(ADJS)]
    B_d = nc.dram_tensor("B", [4, 3, 128, ES], DT, kind="ExternalInput")
    bias_d = nc.dram_tensor("biasrep", [128, ES], F32, kind="ExternalInput")
    iota_d = nc.dram_tensor("iota", [128, 128], DT, kind="ExternalInput")
    out_d = nc.dram_tensor("out", [NDP, ES], F32, kind="ExternalOutput")

    gq = [0]  # gather queue rotation counter

    with tile.TileContext(nc) as tc:
        with tc.tile_pool(name="const", bufs=1) as cpool, \
             tc.tile_pool(name="dew", bufs=6) as dewpool, \
             tc.tile_pool(name="idx", bufs=6) as idxpool, \
             tc.tile_pool(name="st", bufs=STBUFS) as stpool, \
             tc.tile_pool(name="msg", bufs=MSGBUFS) as msgpool, \
             tc.tile_pool(name="ysb", bufs=10) as ypool, \
             tc.tile_pool(name="yT", bufs=13) as ytpool, \
             tc.tile_pool(name="outsb", bufs=2) as opool, \
             tc.tile_pool(name="psagg", bufs=3, space="PSUM") as ps_agg, \
             tc.tile_pool(name="pstp", bufs=2, space="PSUM") as ps_tp, \
             tc.tile_pool(name="psout", bufs=2, space="PSUM") as ps_out:

            B_t = cpool.tile([128, 4, 3, ES], DT)
            nc.sync.dma_start(out=B_t[:], in_=B_d.ap().rearrange("a k p e -> p a k e"))
            bias_t = cpool.tile([128, ES], F32)
            nc.sync.dma_start(out=bias_t[:], in_=bias_d.ap())
            iota_t = cpool.tile([128, 128], DT)
            nc.sync.dma_start(out=iota_t[:], in_=iota_d.ap())
            ident = cpool.tile([128, 128], DT)
            make_identity(nc, ident[:])

            ys = {}

            def emit_agg(w):
                for ai in range(4):
                    nch = int(ncws[ai][w])
                    choff = choffs[ai][w]
                    dew_t = dewpool.tile([128, 2, max_nch], DT, tag="dew")
                    nc.sync.dma_start(out=dew_t[:, :, :nch],
                                      in_=dew_d[ai].ap()[:, :, choff:choff + nch])
                    idx_t = idxpool.tile([128, max_nch * 8], mybir.dt.int16,
                                         tag="idx")
                    nc.sync.dma_start(
                        out=idx_t[:, :nch * 8],
                        in_=gi_d[ai].ap()[:, choff * 8:(choff + nch) * 8])
                    st_t = stpool.tile([128, max_nch, WIN], XDT, tag="st")
                    if "nostbuild" in ablate:
                        nc.vector.tensor_copy(
                            out=st_t[:, 0, :], in_=iota_t[:])
                    else:
                        dlc_b = dew_t[:, 0, :nch].rearrange("p (n o) -> p n o", o=1) \
                            .to_broadcast([128, nch, WIN])
                        iota_b = iota_t[:].rearrange("p (o i) -> p o i", o=1) \
                            .to_broadcast([128, nch, WIN])
                        ewc_b = dew_t[:, 1, :nch].rearrange("p (n o) -> p n o", o=1) \
                            .to_broadcast([128, nch, WIN])
                        nc.vector.tensor_tensor(out=st_t[:, :nch, :], in0=dlc_b,
                                                in1=iota_b,
                                                op=mybir.AluOpType.is_equal)
                        nc.vector.tensor_tensor(out=st_t[:, :nch, :],
                                                in0=st_t[:, :nch, :], in1=ewc_b,
                                                op=mybir.AluOpType.mult)
                    msg = msgpool.tile([128, max_nch, XROW], XDT, tag="msg")
                    if "nogather" not in ablate:
                        nc.gpsimd.dma_gather(
                            msg[:, :nch, :], x_d.ap(), idx_t[:, :nch * 8],
                            nch * 128, nch * 128, XROW, elem_step=XROW,
                            single_packet=False, queue_num=gq[0] % 4)
                        gq[0] += 1
                    else:
                        nc.gpsimd.memset(msg[:], 0.0)
                    ps = ps_agg.tile([128, ES], F32, tag="agg")
                    if "noaggmm" in ablate:
                        nc.tensor.matmul(ps[:], lhsT=st_t[:, 0, :],
                                         rhs=msg[:, 0, :ES], start=True, stop=True)
                    else:
                        for ch in range(nch):
                            nc.tensor.matmul(ps[:], lhsT=st_t[:, ch, :],
                                             rhs=msg[:, ch, :ES],
                                             start=(ch == 0), stop=(ch == nch - 1))
                    ysb = ypool.tile([128, ES], DT, tag="y")
                    nc.scalar.copy(out=ysb[:], in_=ps[:])
                    ys[(w, ai)] = ysb

            def emit_wphase(w):
                if "nowphase" in ablate:
                    for ai in range(4):
                        del ys[(w, ai)]
                    return
                yTs = []
                for ai in range(4):
                    for k in range(3):
                        pst = ps_tp.tile([128, 128], DT, tag="tp")
                        nc.tensor.transpose(
                            pst[:], ys[(w, ai)][:, 128 * k:128 * (k + 1)],
                            ident[:])
                        yT = ytpool.tile([128, 128], DT, tag="yT")
                        if (ai * 3 + k) % 2 == 0:
                            nc.vector.tensor_copy(out=yT[:], in_=pst[:])
                        else:
                            nc.scalar.copy(out=yT[:], in_=pst[:])
                        yTs.append(yT)
                pso = ps_out.tile([128, ES], F32, tag="wout")
                for i, yT in enumerate(yTs):
                    ai, k = divmod(i, 3)
                    nc.tensor.matmul(pso[:], lhsT=yT[:], rhs=B_t[:, ai, k, :],
                                     start=(i == 0), stop=(i == 11))
                outsb = opool.tile([128, ES], F32, tag="out")
                nc.vector.tensor_tensor(out=outsb[:], in0=pso[:], in1=bias_t[:],
                                        op=mybir.AluOpType.add)
                nc.sync.dma_start(out=out_d.ap()[128 * w:128 * (w + 1), :],
                                  in_=outsb[:])
                for ai in range(4):
                    del ys[(w, ai)]

            for _rep in range(rep):
                for w in range(NWIN):
                    emit_agg(w)
                    if w > 0:
                        emit_wphase(w - 1)
                emit_wphase(NWIN - 1)

    nc.compile()
    _CACHE[key] = nc
    return nc


def _host_prep(x, Ws, bias, eis, ews):
    xr = np.asarray(x).astype(np.float32).transpose(0, 2, 1).reshape(N, ES)
    if X8:
        x_rows = np.zeros((N, XROW), ml_dtypes.float8_e4m3)
        x_rows[:, :ES] = xr.astype(ml_dtypes.float8_e4m3)
        x_rows = np.ascontiguousarray(x_rows)
    else:
        x_rows = np.ascontiguousarray(xr).astype(bf16)
    ncws, gis, dlcs = [], [], []
    for a in ADJS:
        pc = _prep_edges(np.asarray(eis[a]), np.asarray(ews[a]))
        ncw, gi, dew = _build_adj_inputs(pc)
        ncws.append(tuple(int(v) for v in ncw))
        gis.append(gi)
        dlcs.append(dew)
    B = _build_B(Ws)
    bias_rep = np.ascontiguousarray(
        np.tile(np.repeat(np.asarray(bias).astype(np.float32), T)[None, :], (128, 1)))
    iota = np.ascontiguousarray(
        np.tile(np.arange(128, dtype=np.float32)[None, :], (128, 1))).astype(bf16)
    in_maps = []
    for k in range(NCORES):
        m = {"xrows": x_rows, "B": B, "biasrep": bias_rep, "iota": iota}
        for ai, a in enumerate(ADJS):
            m[f"gi_{a}"] = gis[ai][k]
            m[f"dew_{a}"] = dlcs[ai][k]
        in_maps.append(m)
    return tuple(ncws), in_maps


def kernel(x, W_fwd1, W_fwd2, W_bck1, W_bck2, bias,
           ew_fwd1, ew_fwd2, ew_bck1, ew_bck2,
           ei_fwd1, ei_fwd2, ei_bck1, ei_bck2):
    from concourse.bass_utils import run_bass_kernel_spmd

    x = np.asarray(x)
    eis = dict(fwd1=ei_fwd1, fwd2=ei_fwd2, bck1=ei_bck1, bck2=ei_bck2)
    ews = dict(fwd1=ew_fwd1, fwd2=ew_fwd2, bck1=ew_bck1, bck2=ew_bck2)
    Ws = [W_fwd1, W_fwd2, W_bck1, W_bck2]

    ncws, in_maps = _host_prep(x, Ws, bias, eis, ews)
    nc = _get_program(ncws)

    res = run_bass_kernel_spmd(nc, in_maps, core_ids=list(range(NCORES)))
    global LAST_RESULTS, LAST_NC, LAST_INMAPS, LAST_NCWS
    LAST_RESULTS = res
    LAST_NC = nc
    LAST_INMAPS = in_maps
    LAST_NCWS = ncws

    out = np.empty((N, C, T), np.float32)
    for k in range(NCORES):
        shard = res.results[k]["out"][:ND]           # [3750, 384], phi'=c*12+t
        out[k * ND:(k + 1) * ND] = shard.reshape(ND, C, T)
    return out
